# revision 14
# baseline (speedup 1.0000x reference)
"""MoE grouped-GEMM (FMoELinear) on 8 trn2 NeuronCores.

Strategy (expert parallelism):
  - 32 experts, 8 cores -> 4 experts per core.
  - Tokens arrive pre-sorted by expert; host pads each expert's segment to a
    fixed per-expert capacity CAP (multiple of CHUNK) and ships each core a
    transposed activation panel xt[256, 4*CAP] plus its 4 expert weights
    wt[256, 4*256] laid out as [in_feat, expert*256 + out_feat].
  - Device computes yt[o, t] = sum_i W[e][o, i] * x[t, i] per expert with the
    weight stationary in the PE array:
        lhsT = wt[i_chunk, e*256 + oc*128 : +128]   (128 x 128, stationary)
        rhs  = xt tile    [i_chunk, token span]     (128 x 512, moving)
    accumulating the two i-chunks into PSUM, then copies PSUM->SBUF->HBM.
  - Host gathers the non-padded columns back into token order.

The program is identical on all 8 cores (pure SPMD data parallelism); only the
input data differs. All routing logic runs on the host using the runtime
fwd_expert_count values.
"""

import os

import numpy as np

import concourse.bacc as bacc
import concourse.mybir as mybir
import concourse.tile as tile
from concourse.bass_utils import run_bass_kernel_spmd

NCORES = 8
D = 256  # in/out feature dim
EPC = 4  # experts per core
CHUNK = int(os.environ.get("BASSMOE_CHUNK", "1024"))  # token-span per load
CAPGRAN = 128  # capacity granularity (pad each expert to a multiple of this)

# matmul input dtype: "f32" (exact, 4 cyc/row), "f32r" (1 cyc/row), "bf16"
MM_DT = os.environ.get("BASSMOE_MM_DT", "f32r")
Y_GPSIMD = bool(int(os.environ.get("BASSMOE_Y_GPSIMD", "0")))
WARM = bool(int(os.environ.get("BASSMOE_WARM", "0")))

# observability for test harness
last_exec_time_ns = None
last_results = None

_prog_cache = {}


def _dtypes():
    if MM_DT == "f32":
        return mybir.dt.float32, np.float32
    if MM_DT == "f32r":
        return mybir.dt.float32r, np.float32
    if MM_DT == "bf16":
        import ml_dtypes

        return mybir.dt.bfloat16, np.dtype(ml_dtypes.bfloat16)
    raise ValueError(MM_DT)


def _chunk_offsets(cap: int):
    """(offset, width) chunks covering [0, cap), width <= CHUNK."""
    out = []
    off = 0
    while off < cap:
        w = min(CHUNK, cap - off)
        out.append((off, w))
        off += w
    return out


def _splits(width: int):
    """(offset, width) matmul spans <= 512 covering [0, width)."""
    out = []
    off = 0
    while off < width:
        w = min(512, width - off)
        out.append((off, w))
        off += w
    return out


def _build_program(cap: int):
    """Build the SPMD Bass program for per-expert capacity `cap` tokens."""
    dt_in, _ = _dtypes()
    width = EPC * cap

    nc = bacc.Bacc(
        "TRN2",
        target_bir_lowering=False,
        debug=False,
        enable_asserts=False,
        num_devices=NCORES,
    )
    xt = nc.dram_tensor("xt", [D, width], dt_in, kind="ExternalInput").ap()
    wt = nc.dram_tensor("wt", [D, EPC * D], dt_in, kind="ExternalInput").ap()
    yt = nc.dram_tensor("yt", [D, width], mybir.dt.float32, kind="ExternalOutput").ap()

    with tile.TileContext(nc) as tc:
        with (
            tc.tile_pool(name="w", bufs=1) as wpool,
            tc.tile_pool(name="x", bufs=6) as xpool,
            tc.tile_pool(name="y", bufs=6) as ypool,
            tc.tile_pool(name="ps", bufs=7 if WARM else 8, space="PSUM") as pspool,
            tc.tile_pool(name="pw", bufs=1, space="PSUM") as pwpool,
        ):
            # stationary weights for the whole kernel: two i-chunks
            # (loaded via gpsimd so they don't head-of-line block the x loads)
            w0 = wpool.tile([128, EPC * D], dt_in, tag="w0")
            w1 = wpool.tile([128, EPC * D], dt_in, tag="w1")
            nc.gpsimd.dma_start(out=w0[:], in_=wt[0:128, :])
            nc.gpsimd.dma_start(out=w1[:], in_=wt[128:256, :])

            chidx = 0
            for e in range(EPC):
                for coff, cw in _chunk_offsets(cap):
                    chidx += 1
                    t0 = e * cap + coff
                    x0 = xpool.tile([128, CHUNK], dt_in, tag="x0")
                    x1 = xpool.tile([128, CHUNK], dt_in, tag="x1")
                    nc.sync.dma_start(out=x0[:, :cw], in_=xt[0:128, t0 : t0 + cw])
                    nc.sync.dma_start(out=x1[:, :cw], in_=xt[128:256, t0 : t0 + cw])
                    if WARM:
                        # tiny matmul tied to this chunk's load keeps the PE's
                        # HAM activity window non-idle (K=8/8, 2.4 GHz)
                        pw = pwpool.tile([128, 8], mybir.dt.float32, tag="warm")
                        nc.tensor.matmul(
                            pw[:], w0[:, 0:128], x0[:, 0:8], start=True, stop=True
                        )
                    for oc in range(2):
                        col = e * D + oc * 128
                        ysb = ypool.tile([128, CHUNK], mybir.dt.float32, tag="y")
                        for soff, sw in _splits(cw):
                            ps = pspool.tile([128, 512], mybir.dt.float32, tag="ps")
                            nc.tensor.matmul(
                                ps[:, :sw],
                                w0[:, col : col + 128],
                                x0[:, soff : soff + sw],
                                start=True,
                                stop=False,
                            )
                            nc.tensor.matmul(
                                ps[:, :sw],
                                w1[:, col : col + 128],
                                x1[:, soff : soff + sw],
                                start=False,
                                stop=True,
                            )
                            nc.vector.tensor_copy(
                                ysb[:, soff : soff + sw], ps[:, :sw]
                            )
                        # stores on the ACT HWDGE ring, loads on the SP ring
                        st_eng = (
                            nc.gpsimd if (Y_GPSIMD and (chidx + oc) % 2) else nc.scalar
                        )
                        st_eng.dma_start(
                            out=yt[oc * 128 : (oc + 1) * 128, t0 : t0 + cw],
                            in_=ysb[:, :cw],
                        )
    nc.compile()
    return nc


def kernel(inp, weight, fwd_expert_count, capacity):
    global last_exec_time_ns, last_results

    inp = np.asarray(inp)
    weight = np.asarray(weight)
    counts = np.asarray(fwd_expert_count).astype(np.int64)
    T, d_in = inp.shape
    E = weight.shape[0]
    assert d_in == D and E == NCORES * EPC
    assert int(counts.sum()) == T, "counts must cover all tokens"

    ends = np.cumsum(counts)
    starts = ends - counts
    cap = max(CAPGRAN, int(-(-int(counts.max()) // CAPGRAN)) * CAPGRAN)
    width = EPC * cap

    _, np_in = _dtypes()

    # host-side scatter: transpose once, then contiguous row-slice copies
    xt_full = np.ascontiguousarray(inp.T)  # [D, T] float32
    if np_in != np.float32:
        xt_full = xt_full.astype(np_in)

    in_maps = []
    for dcore in range(NCORES):
        xt = np.zeros((D, width), dtype=np_in)
        for j in range(EPC):
            e = dcore * EPC + j
            s, c = int(starts[e]), int(counts[e])
            xt[:, j * cap : j * cap + c] = xt_full[:, s : s + c]
        wl = weight[dcore * EPC : (dcore + 1) * EPC]  # [EPC, out, in]
        wt = np.ascontiguousarray(wl.transpose(2, 0, 1).reshape(D, EPC * D))
        in_maps.append({"xt": xt, "wt": wt.astype(np_in)})

    key = (cap, MM_DT)
    if key not in _prog_cache:
        _prog_cache[key] = _build_program(cap)
    nc = _prog_cache[key]

    trace = bool(int(os.environ.get("BASSMOE_TRACE", "0")))
    res = run_bass_kernel_spmd(nc, in_maps, list(range(NCORES)), trace=trace)
    last_exec_time_ns = res.exec_time_ns
    last_results = res

    # gather back to token order
    out_t = np.empty((D, T), dtype=np.float32)
    for dcore in range(NCORES):
        yt = res.results[dcore]["yt"]
        for j in range(EPC):
            e = dcore * EPC + j
            s, c = int(starts[e]), int(counts[e])
            out_t[:, s : s + c] = yt[:, j * cap : j * cap + c]
    return np.ascontiguousarray(out_t.T)
